# revision 4
# baseline (speedup 1.0000x reference)
"""Trainium2 Bass kernel for a 2-layer bidirectional GRU + dense vocab head.

Problem shapes (hardcoded):
  x [32, 512, 512] f32 -> feats [32, 512, 512] f32, out [32, 512, 6000] f32

Distribution across 8 NeuronCores: direction-parallel x batch-parallel.
  core c: direction d = c // 4, batch shard s = c % 4 (examples 8s..8s+8).
  Each core runs its direction's GRU chains for its 8 examples in a
  feature-transposed layout (feature on partitions, batch on free dim).
  Between layers, forward/backward partner cores exchange their (time-
  flipped) layer outputs with a pairwise AllGather; a runtime branch on
  the collective rank picks the partner's buffer. The final dense is
  split: rank-0 cores compute local examples 0:4, rank-1 cores 4:8.
  Time reversal for backward cores is handled on the host (inputs fed
  time-flipped, outputs flipped back).

All matmuls run in bf16 (f32 PSUM accumulation); GRU state is bf16;
gate preactivation x-parts are folded into PSUM via identity matmuls.
"""
import os
import sys

sys.path.insert(0, "/opt/trn_rl_repo")

import numpy as np
import ml_dtypes

import concourse.bacc as bacc
import concourse.mybir as mybir
import concourse.tile as tile
from concourse.bass_utils import run_bass_kernel_spmd

bf = ml_dtypes.bfloat16
DT = mybir.dt
AF = mybir.ActivationFunctionType
ALU = mybir.AluOpType

P = 128
B, T, F, U, V = 32, 512, 512, 256, 6000
BL = 8            # examples per core
BH = 4            # dense half per core
NC = 8
GROUPS = [[0, 4], [1, 5], [2, 6], [3, 7]]
VT = [512] * 11 + [368]   # V tiles

_CACHE = {}
LAST_EXEC_NS = None


def _emit_xproj(nc, gxc, rhs_tiles, wgx, wcx, bgt, bct, psx):
    """x-projections for one layer into gxc [P, T, 6, BL] bf16.

    rhs_tiles: list of 4 callables k -> AP [P, 64, BL] for a given t0 block.
    wgx [P, 4, 4, P], wcx [P, 4, 2, P]; biases [P, 4], [P, 2].
    """
    nblk = T // 64
    for blk in range(nblk):
        t0 = blk * 64
        for m in range(6):
            ps = psx.tile([P, 64, BL], DT.float32, tag="psx", name=f"psx_{id(gxc)}_{blk}_{m}")
            for k in range(4):
                w = wgx[:, k, m, :] if m < 4 else wcx[:, k, m - 4, :]
                nc.tensor.matmul(ps[:], w, rhs_tiles[k](t0),
                                 start=(k == 0), stop=(k == 3))
            bias = bgt[:, m:m + 1] if m < 4 else bct[:, m - 4:m - 3]
            nc.scalar.activation(gxc[:, t0:t0 + 64, m, :], ps[:], AF.Identity,
                                 bias=bias)


def _emit_scan(nc, lbl, gxc, hbuf, flip, wgh, wch, idt, psr, psu, psc, work):
    """v5 GRU scan. gxc [P,T,6,BL] bf16; hbuf [P,T+1,2,BL]; flip [P,2,T,BL]."""
    nc.vector.memset(hbuf[:, 0, :, :], 0.0)
    pgr = {}
    pgu = {}
    pc = {}

    def imm_r(t):
        tl = psr.tile([P, 2, BL], DT.float32, tag="pgr", name=f"pgr{lbl}_{t}")
        pgr[t] = tl
        nc.tensor.matmul(tl[:], idt[:], gxc[:, t, 0:2, :], start=True,
                         stop=False, skip_group_check=True)

    def imm_u(t):
        tl = psu.tile([P, 2, BL], DT.float32, tag="pgu", name=f"pgu{lbl}_{t}")
        pgu[t] = tl
        nc.tensor.matmul(tl[:], idt[:], gxc[:, t, 2:4, :], start=True,
                         stop=False, skip_group_check=True)

    def imm_c(t):
        tl = psc.tile([P, 2, BL], DT.float32, tag="pc", name=f"pc{lbl}_{t}")
        pc[t] = tl
        nc.tensor.matmul(tl[:], idt[:], gxc[:, t, 4:6, :], start=True,
                         stop=False, skip_group_check=True)

    imm_r(0); imm_u(0); imm_c(0)
    for t in range(T):
        if t + 1 < T:
            imm_r(t + 1); imm_u(t + 1)
        for m in (0, 1):
            for k in (0, 1):
                nc.tensor.matmul(pgr[t][:, m, :], wgh[:, k, m, :],
                                 hbuf[:, t, k, :], start=False,
                                 stop=(k == 1), skip_group_check=True)
        for m in (0, 1):
            for k in (0, 1):
                nc.tensor.matmul(pgu[t][:, m, :], wgh[:, k, m + 2, :],
                                 hbuf[:, t, k, :], start=False,
                                 stop=(k == 1), skip_group_check=True)
        ru_r = work.tile([P, 2, BL], DT.bfloat16, tag="ru_r", name=f"rur{lbl}_{t}")
        nc.scalar.activation(ru_r[:], pgr[t][:], AF.Sigmoid)
        rh = work.tile([P, 2, BL], DT.bfloat16, tag="rh", name=f"rh{lbl}_{t}")
        nc.vector.tensor_tensor(rh[:], ru_r[:], hbuf[:, t, :, :], ALU.mult)
        if t + 1 < T:
            imm_c(t + 1)
        for m in (0, 1):
            for k in (0, 1):
                nc.tensor.matmul(pc[t][:, m, :], wch[:, k, m, :],
                                 rh[:, k, :], start=False,
                                 stop=(k == 1), skip_group_check=True)
        u = work.tile([P, 2, BL], DT.float32, tag="u", name=f"u{lbl}_{t}")
        nc.scalar.activation(u[:], pgu[t][:], AF.Sigmoid)
        c = work.tile([P, 2, BL], DT.float32, tag="c", name=f"c{lbl}_{t}")
        nc.scalar.activation(c[:], pc[t][:], AF.Tanh)
        d = work.tile([P, 2, BL], DT.float32, tag="d", name=f"d{lbl}_{t}")
        nc.vector.tensor_tensor(d[:], hbuf[:, t, :, :], c[:], ALU.subtract)
        e = work.tile([P, 2, BL], DT.float32, tag="e", name=f"e{lbl}_{t}")
        nc.vector.tensor_tensor(e[:], d[:], u[:], ALU.mult)
        nc.vector.tensor_tensor(hbuf[:, t + 1, :, :], c[:], e[:], ALU.add)
        nc.gpsimd.tensor_copy(flip[:, :, T - 1 - t, :], hbuf[:, t + 1, :, :])
        del pgr[t], pgu[t], pc[t]


def build_program():
    nc = bacc.Bacc("TRN2", target_bir_lowering=False, debug=False, num_devices=NC)
    ein = lambda n, s, d: nc.dram_tensor(n, s, d, kind="ExternalInput")
    xin = ein("xin", [BL, T, F], DT.float32)
    wgx0 = ein("wgx0", [P, 4, 4, P], DT.bfloat16)
    wcx0 = ein("wcx0", [P, 4, 2, P], DT.bfloat16)
    wgh0 = ein("wgh0", [P, 2, 4, P], DT.bfloat16)
    wch0 = ein("wch0", [P, 2, 2, P], DT.bfloat16)
    bg0 = ein("bg0", [P, 4], DT.float32)
    bc0 = ein("bc0", [P, 2], DT.float32)
    wgx1 = ein("wgx1", [P, 4, 4, P], DT.bfloat16)
    wcx1 = ein("wcx1", [P, 4, 2, P], DT.bfloat16)
    wgh1 = ein("wgh1", [P, 2, 4, P], DT.bfloat16)
    wch1 = ein("wch1", [P, 2, 2, P], DT.bfloat16)
    bg1 = ein("bg1", [P, 4], DT.float32)
    bc1 = ein("bc1", [P, 2], DT.float32)
    woutp = ein("woutp", [P, 4, V], DT.bfloat16)
    identp = ein("identp", [P, P], DT.bfloat16)
    feats_out = nc.dram_tensor("feats_out", [BH, T, 2 * U], DT.float32,
                               kind="ExternalOutput")
    dense_out = nc.dram_tensor("dense_out", [BH, T, V], DT.float32,
                               kind="ExternalOutput")

    with tile.TileContext(nc) as tc:
        with tc.tile_pool(name="wp", bufs=1) as wp, \
             tc.tile_pool(name="gxcp", bufs=1) as gxcp, \
             tc.tile_pool(name="h0p", bufs=1) as h0p, \
             tc.tile_pool(name="work", bufs=4) as work, \
             tc.tile_pool(name="dramp", bufs=1, space="DRAM") as dramp:
            # persistent weights
            wgx0t = wp.tile([P, 4, 4, P], DT.bfloat16)
            wcx0t = wp.tile([P, 4, 2, P], DT.bfloat16)
            wgh0t = wp.tile([P, 2, 4, P], DT.bfloat16)
            wch0t = wp.tile([P, 2, 2, P], DT.bfloat16)
            wgx1t = wp.tile([P, 4, 4, P], DT.bfloat16)
            wcx1t = wp.tile([P, 4, 2, P], DT.bfloat16)
            wgh1t = wp.tile([P, 2, 4, P], DT.bfloat16)
            wch1t = wp.tile([P, 2, 2, P], DT.bfloat16)
            bg0t = wp.tile([P, 4], DT.float32)
            bc0t = wp.tile([P, 2], DT.float32)
            bg1t = wp.tile([P, 4], DT.float32)
            bc1t = wp.tile([P, 2], DT.float32)
            idt = wp.tile([P, P], DT.bfloat16)
            for dst, src in ((wgx0t, wgx0), (wcx0t, wcx0), (wgh0t, wgh0),
                             (wch0t, wch0), (wgx1t, wgx1), (wcx1t, wcx1),
                             (wgh1t, wgh1), (wch1t, wch1), (bg0t, bg0),
                             (bc0t, bc0), (bg1t, bg1), (bc1t, bc1),
                             (idt, identp)):
                nc.sync.dma_start(dst[:], src[:])

            gxc = gxcp.tile([P, T, 6, BL], DT.bfloat16)   # shared L0/L1
            hbuf0 = h0p.tile([P, T + 1, 2, BL], DT.bfloat16)

            rank = nc.sync.cc_rank(GROUPS)

            # ---- phase 0: transpose x to xT [P, 4, T, BL] bf16 ----
            with tc.tile_pool(name="xtp", bufs=1) as xtp, \
                 tc.tile_pool(name="fl0p", bufs=1) as fl0p:
                flip0 = fl0p.tile([P, 2, T, BL], DT.bfloat16)
                xT = xtp.tile([P, 4, T, BL], DT.bfloat16)
                with tc.tile_pool(name="xload", bufs=4) as xload, \
                     tc.tile_pool(name="pst", bufs=4, space="PSUM") as pst:
                    for ex in range(BL):
                        for tt in range(4):
                            for ff in range(4):
                                xf = xload.tile([P, P], DT.float32, tag="xf",
                                                name=f"xf{ex}_{tt}_{ff}")
                                nc.sync.dma_start(
                                    xf[:], xin[ex, tt * P:(tt + 1) * P,
                                               ff * P:(ff + 1) * P])
                                xb = xload.tile([P, P], DT.bfloat16, tag="xb",
                                                name=f"xb{ex}_{tt}_{ff}")
                                nc.vector.tensor_copy(xb[:], xf[:])
                                pt = pst.tile([P, P], DT.bfloat16, tag="pt",
                                              name=f"pt{ex}_{tt}_{ff}")
                                nc.tensor.transpose(pt[:], xb[:], idt[:])
                                nc.vector.tensor_copy(
                                    xT[:, ff, tt * P:(tt + 1) * P, ex], pt[:])

                # ---- phase 1: L0 x-projections ----
                with tc.tile_pool(name="psx0", bufs=2, space="PSUM") as psx0:
                    rhs0 = [
                        (lambda k: (lambda t0: xT[:, k, t0:t0 + 64, :]))(k)
                        for k in range(4)
                    ]
                    _emit_xproj(nc, gxc, rhs0, wgx0t, wcx0t, bg0t, bc0t, psx0)

                # ---- phase 2: L0 scan ----
                with tc.tile_pool(name="psr0", bufs=3, space="PSUM") as psr, \
                     tc.tile_pool(name="psu0", bufs=3, space="PSUM") as psu, \
                     tc.tile_pool(name="psc0", bufs=2, space="PSUM") as psc:
                    _emit_scan(nc, "a", gxc, hbuf0, flip0, wgh0t, wch0t, idt,
                               psr, psu, psc, work)

                # ---- phase 3: exchange L0 ----
                cc0_in = dramp.tile([P, 2, T, BL], DT.bfloat16)
                cc0_out = dramp.tile([2, P, 2, T, BL], DT.bfloat16)
                nc.sync.dma_start(cc0_in[:], flip0[:])
                nc.gpsimd.collective_compute(
                    "AllGather", ALU.bypass, replica_groups=GROUPS,
                    ins=[cc0_in[:]], outs=[cc0_out[:]])

            with tc.tile_pool(name="r0p", bufs=1) as r0p, \
                 tc.tile_pool(name="h1p", bufs=1) as h1p:
                recv0 = r0p.tile([P, 2, T, BL], DT.bfloat16)
                with tc.If(rank < 1) as cmp:
                    nc.sync.dma_start(recv0[:], cc0_out[1])
                with cmp.Else():
                    nc.sync.dma_start(recv0[:], cc0_out[0])

                hbuf1 = h1p.tile([P, T + 1, 2, BL], DT.bfloat16)

                # ---- phase 4: L1 x-projections ----
                with tc.tile_pool(name="psx1", bufs=2, space="PSUM") as psx1:
                    def mk_rhs1(k):
                        if k < 2:
                            return lambda t0: hbuf0[:, 1 + t0:1 + t0 + 64, k, :]
                        return lambda t0: recv0[:, k - 2, t0:t0 + 64, :]
                    rhs1 = [mk_rhs1(k) for k in range(4)]
                    _emit_xproj(nc, gxc, rhs1, wgx1t, wcx1t, bg1t, bc1t, psx1)

                # ---- phase 5: L1 scan ----
                with tc.tile_pool(name="fl1p", bufs=1) as fl1p:
                    flip1 = fl1p.tile([P, 2, T, BL], DT.bfloat16)
                    with tc.tile_pool(name="psr1", bufs=3, space="PSUM") as psr, \
                         tc.tile_pool(name="psu1", bufs=3, space="PSUM") as psu, \
                         tc.tile_pool(name="psc1", bufs=2, space="PSUM") as psc:
                        _emit_scan(nc, "b", gxc, hbuf1, flip1, wgh1t, wch1t,
                                   idt, psr, psu, psc, work)

                    # ---- phase 6: exchange L1 ----
                    cc1_in = dramp.tile([P, 2, T, BL], DT.bfloat16)
                    cc1_out = dramp.tile([2, P, 2, T, BL], DT.bfloat16)
                    nc.sync.dma_start(cc1_in[:], flip1[:])
                    nc.gpsimd.collective_compute(
                        "AllGather", ALU.bypass, replica_groups=GROUPS,
                        ins=[cc1_in[:]], outs=[cc1_out[:]])

                with tc.tile_pool(name="r1p", bufs=1) as r1p, \
                     tc.tile_pool(name="stgp", bufs=1) as stgp:
                    recv1 = r1p.tile([P, 2, T, BL], DT.bfloat16)
                    stage = stgp.tile([P, 4, T, BH], DT.bfloat16)
                    with tc.If(rank < 1) as cmp:
                        nc.sync.dma_start(recv1[:], cc1_out[1])
                        for k in (0, 1):
                            nc.sync.dma_start(
                                stage[:, k, :, :], hbuf1[:, 1:, k, 0:BH])
                        for k in (0, 1):
                            nc.sync.dma_start(
                                stage[:, 2 + k, :, :], recv1[:, k, :, 0:BH])
                    with cmp.Else():
                        nc.sync.dma_start(recv1[:], cc1_out[0])
                        for k in (0, 1):
                            nc.sync.dma_start(
                                stage[:, k, :, :], hbuf1[:, 1:, k, BH:BL])
                        for k in (0, 1):
                            nc.sync.dma_start(
                                stage[:, 2 + k, :, :], recv1[:, k, :, BH:BL])

                    # ---- phase 7: feats output (transpose stage) ----
                    with tc.tile_pool(name="ftp", bufs=4, space="PSUM") as ftp, \
                         tc.tile_pool(name="fsb", bufs=4) as fsb:
                        for ex in range(BH):
                            for kt in range(4):
                                for tt in range(4):
                                    pt = ftp.tile([P, P], DT.bfloat16, tag="fpt",
                                                  name=f"fpt{ex}_{kt}_{tt}")
                                    nc.tensor.transpose(
                                        pt[:],
                                        stage[:, kt, tt * P:(tt + 1) * P, ex],
                                        idt[:])
                                    sb = fsb.tile([P, P], DT.float32, tag="fsb",
                                                  name=f"fsb{ex}_{kt}_{tt}")
                                    nc.vector.tensor_copy(sb[:], pt[:])
                                    nc.sync.dma_start(
                                        feats_out[ex, tt * P:(tt + 1) * P,
                                                  kt * P:(kt + 1) * P], sb[:])

                    # ---- phase 8: dense ----
                    with tc.tile_pool(name="wob", bufs=3) as wob, \
                         tc.tile_pool(name="pd", bufs=6, space="PSUM") as pd, \
                         tc.tile_pool(name="dsb", bufs=4) as dsb:
                        for mt in range(16):
                            # M-tile = 32 timesteps x 4 examples
                            t0 = mt * 32
                            for vb in range(2):
                                vts = list(range(vb * 6, min(vb * 6 + 6, 12)))
                                pss = {}
                                for k in range(4):
                                    wo = wob.tile([P, 3072], DT.bfloat16,
                                                  tag="wo",
                                                  name=f"wo{mt}_{vb}_{k}")
                                    v0 = vb * 6 * 512
                                    nv_blk = sum(VT[vt] for vt in vts)
                                    nc.sync.dma_start(
                                        wo[:, 0:nv_blk],
                                        woutp[:, k, v0:v0 + nv_blk])
                                    off = 0
                                    for vt in vts:
                                        nv = VT[vt]
                                        if k == 0:
                                            pss[vt] = pd.tile(
                                                [P, 512], DT.float32, tag="pd",
                                                name=f"pd{mt}_{vt}")
                                        nc.tensor.matmul(
                                            pss[vt][:, 0:nv],
                                            stage[:, k, t0:t0 + 32, :],
                                            wo[:, off:off + nv],
                                            start=(k == 0), stop=(k == 3))
                                        off += nv
                                for vt in vts:
                                    nv = VT[vt]
                                    v0 = vt * 512
                                    sb = dsb.tile([P, 512], DT.float32,
                                                  tag="dsb",
                                                  name=f"dsb{mt}_{vt}")
                                    nc.vector.tensor_copy(sb[:, 0:nv],
                                                          pss[vt][:, 0:nv])
                                    nc.sync.dma_start(
                                        dense_out[:, t0:t0 + 32,
                                                  v0:v0 + nv].transpose(
                                                      [1, 0, 2]),
                                        sb[:, 0:nv])
    nc.compile()
    return nc


def _pack_w(w, nk, nm):
    """w [nk*128, nm*128] -> [P, nk, nm, P]"""
    return np.ascontiguousarray(
        w.reshape(nk, P, nm, P).transpose(1, 0, 2, 3)).astype(bf)


def _pack_bias(b, nm):
    return np.ascontiguousarray(b.reshape(nm, P).T).astype(np.float32)


def _prep_inputs(x, Wg, bg, Wc, bc, Wout, bout):
    """Build the 8 per-core input maps."""
    in_maps = []
    ident = np.eye(P).astype(bf)
    for c in range(NC):
        d, s = c // 4, c % 4
        ex = slice(s * BL, (s + 1) * BL)
        xs = x[ex]
        if d == 1:
            xs = xs[:, ::-1, :]
        m = {"xin": np.ascontiguousarray(xs, np.float32), "identp": ident}
        # layer 0
        Wg0, Wc0 = Wg[0, d], Wc[0, d]
        m["wgx0"] = _pack_w(Wg0[:F], 4, 4)
        m["wgh0"] = _pack_w(Wg0[F:], 2, 4)
        m["wcx0"] = _pack_w(Wc0[:F], 4, 2)
        m["wch0"] = _pack_w(Wc0[F:], 2, 2)
        m["bg0"] = _pack_bias(bg[0, d], 4)
        m["bc0"] = _pack_bias(bc[0, d], 2)
        # layer 1: x rows own-first
        Wg1, Wc1 = Wg[1, d], Wc[1, d]
        own, oth = slice(d * U, (d + 1) * U), slice((1 - d) * U, (2 - d) * U)
        Wg1x = np.concatenate([Wg1[:F][own], Wg1[:F][oth]], 0)
        Wc1x = np.concatenate([Wc1[:F][own], Wc1[:F][oth]], 0)
        m["wgx1"] = _pack_w(Wg1x, 4, 4)
        m["wgh1"] = _pack_w(Wg1[F:], 2, 4)
        m["wcx1"] = _pack_w(Wc1x, 4, 2)
        m["wch1"] = _pack_w(Wc1[F:], 2, 2)
        m["bg1"] = _pack_bias(bg[1, d], 4)
        m["bc1"] = _pack_bias(bc[1, d], 2)
        # Wout rows own-first
        Wo = Wout if d == 0 else np.concatenate([Wout[U:], Wout[:U]], 0)
        m["woutp"] = np.ascontiguousarray(
            Wo.reshape(4, P, V).transpose(1, 0, 2)).astype(bf)
        in_maps.append(m)
    return in_maps


def kernel(x, Wg, bg, Wc, bc, Wout, bout, training):
    global LAST_EXEC_NS
    x = np.asarray(x, np.float32)
    Wg = np.asarray(Wg, np.float32)
    bg = np.asarray(bg, np.float32)
    Wc = np.asarray(Wc, np.float32)
    bc = np.asarray(bc, np.float32)
    Wout = np.asarray(Wout, np.float32)
    bout = np.asarray(bout, np.float32)

    if "nc" not in _CACHE:
        _CACHE["nc"] = build_program()
    nc = _CACHE["nc"]
    in_maps = _prep_inputs(x, Wg, bg, Wc, bc, Wout, bout)
    trace = bool(int(os.environ.get("KERNEL_TRACE", "0")))
    if trace:
        sys.path.insert(0, os.path.dirname(os.path.abspath(__file__)))
        import axon_prof  # noqa: F401
    r = run_bass_kernel_spmd(nc, in_maps, list(range(NC)), trace=trace)
    LAST_EXEC_NS = r.exec_time_ns

    feats = np.zeros((B, T, 2 * U), np.float32)
    out = np.zeros((B, T, V), np.float32)
    for c in range(NC):
        d, s = c // 4, c % 4
        fo = r.results[c]["feats_out"]    # [4, T, 512] local-time, own-first
        do = r.results[c]["dense_out"]    # [4, T, V]
        if d == 0:
            exs = range(s * BL, s * BL + BH)
            for i, e in enumerate(exs):
                feats[e] = fo[i]
                out[e] = do[i]
        else:
            exs = range(s * BL + BH, s * BL + BL)
            for i, e in enumerate(exs):
                feats[e] = np.concatenate(
                    [fo[i, ::-1, U:], fo[i, ::-1, :U]], axis=-1)
                out[e] = do[i, ::-1]
    out += bout
    return feats, out


# revision 5
# speedup vs baseline: 1.1043x; 1.1043x over previous
"""Trainium2 Bass kernel for a 2-layer bidirectional GRU + dense vocab head.

Problem shapes (hardcoded):
  x [32, 512, 512] f32 -> feats [32, 512, 512] f32, out [32, 512, 6000] f32

Distribution across 8 NeuronCores: direction-parallel x batch-parallel.
  core c: direction d = c // 4, batch shard s = c % 4 (examples 8s..8s+8).
  Each core runs its direction's GRU chains for its 8 examples in a
  feature-transposed layout (feature on partitions, batch on free dim).
  Between layers, forward/backward partner cores exchange their (time-
  flipped) layer outputs with a pairwise AllGather; a runtime branch on
  the collective rank picks the partner's buffer. The final dense is
  split: rank-0 cores compute local examples 0:4, rank-1 cores 4:8.
  Time reversal for backward cores is handled on the host (inputs fed
  time-flipped, outputs flipped back).

All matmuls run in bf16 (f32 PSUM accumulation); GRU state is bf16;
gate preactivation x-parts are folded into PSUM via identity matmuls.
"""
import os
import sys

sys.path.insert(0, "/opt/trn_rl_repo")

import numpy as np
import ml_dtypes

import concourse.bacc as bacc
import concourse.mybir as mybir
import concourse.tile as tile
from concourse.bass_utils import run_bass_kernel_spmd

bf = ml_dtypes.bfloat16
DT = mybir.dt
AF = mybir.ActivationFunctionType
ALU = mybir.AluOpType

P = 128
B, T, F, U, V = 32, 512, 512, 256, 6000
BL = 8            # examples per core
BH = 4            # dense half per core
NC = 8
GROUPS = [[0, 4], [1, 5], [2, 6], [3, 7]]
VT = [512] * 11 + [368]   # V tiles

_CACHE = {}
LAST_EXEC_NS = None


def _emit_xproj(nc, gxc, rhs_tiles, wgx, wcx, bgt, bct, psx):
    """x-projections for one layer into gxc [P, T, 6, BL] bf16.

    rhs_tiles: list of 4 callables k -> AP [P, 64, BL] for a given t0 block.
    wgx [P, 4, 4, P], wcx [P, 4, 2, P]; biases [P, 4], [P, 2].
    """
    nblk = T // 64
    for blk in range(nblk):
        t0 = blk * 64
        for m in range(6):
            ps = psx.tile([P, 64, BL], DT.float32, tag="psx", name=f"psx_{id(gxc)}_{blk}_{m}")
            for k in range(4):
                w = wgx[:, k, m, :] if m < 4 else wcx[:, k, m - 4, :]
                nc.tensor.matmul(ps[:], w, rhs_tiles[k](t0),
                                 start=(k == 0), stop=(k == 3))
            bias = bgt[:, m:m + 1] if m < 4 else bct[:, m - 4:m - 3]
            nc.scalar.activation(gxc[:, t0:t0 + 64, m, :], ps[:], AF.Identity,
                                 bias=bias)


def _emit_scan(nc, lbl, gxc, hbuf, flip, wgh, wch, idt, psr, psu, psc, work):
    """v5 GRU scan. gxc [P,T,6,BL] bf16; hbuf [P,T+1,2,BL]; flip [P,2,T,BL]."""
    nc.vector.memset(hbuf[:, 0, :, :], 0.0)
    pgr = {}
    pgu = {}
    pc = {}

    def imm_r(t):
        tl = psr.tile([P, 2, BL], DT.float32, tag="pgr", name=f"pgr{lbl}_{t}")
        pgr[t] = tl
        nc.tensor.matmul(tl[:], idt[:], gxc[:, t, 0:2, :], start=True,
                         stop=False, skip_group_check=True)

    def imm_u(t):
        tl = psu.tile([P, 2, BL], DT.float32, tag="pgu", name=f"pgu{lbl}_{t}")
        pgu[t] = tl
        nc.tensor.matmul(tl[:], idt[:], gxc[:, t, 2:4, :], start=True,
                         stop=False, skip_group_check=True)

    def imm_c(t):
        tl = psc.tile([P, 2, BL], DT.float32, tag="pc", name=f"pc{lbl}_{t}")
        pc[t] = tl
        nc.tensor.matmul(tl[:], idt[:], gxc[:, t, 4:6, :], start=True,
                         stop=False, skip_group_check=True)

    imm_r(0); imm_u(0); imm_c(0)
    for t in range(T):
        if t + 1 < T:
            imm_r(t + 1); imm_u(t + 1)
        for m in (0, 1):
            for k in (0, 1):
                nc.tensor.matmul(pgr[t][:, m, :], wgh[:, k, m, :],
                                 hbuf[:, t, k, :], start=False,
                                 stop=(k == 1), skip_group_check=True)
        for m in (0, 1):
            for k in (0, 1):
                nc.tensor.matmul(pgu[t][:, m, :], wgh[:, k, m + 2, :],
                                 hbuf[:, t, k, :], start=False,
                                 stop=(k == 1), skip_group_check=True)
        ru_r = work.tile([P, 2, BL], DT.bfloat16, tag="ru_r", name=f"rur{lbl}_{t}")
        nc.scalar.activation(ru_r[:], pgr[t][:], AF.Sigmoid)
        rh = work.tile([P, 2, BL], DT.bfloat16, tag="rh", name=f"rh{lbl}_{t}")
        nc.vector.tensor_tensor(rh[:], ru_r[:], hbuf[:, t, :, :], ALU.mult)
        if t + 1 < T:
            imm_c(t + 1)
        for m in (0, 1):
            for k in (0, 1):
                nc.tensor.matmul(pc[t][:, m, :], wch[:, k, m, :],
                                 rh[:, k, :], start=False,
                                 stop=(k == 1), skip_group_check=True)
        u = work.tile([P, 2, BL], DT.float32, tag="u", name=f"u{lbl}_{t}")
        nc.scalar.activation(u[:], pgu[t][:], AF.Sigmoid)
        c = work.tile([P, 2, BL], DT.float32, tag="c", name=f"c{lbl}_{t}")
        nc.scalar.activation(c[:], pc[t][:], AF.Tanh)
        d = work.tile([P, 2, BL], DT.float32, tag="d", name=f"d{lbl}_{t}")
        nc.vector.tensor_tensor(d[:], hbuf[:, t, :, :], c[:], ALU.subtract)
        e = work.tile([P, 2, BL], DT.float32, tag="e", name=f"e{lbl}_{t}")
        nc.vector.tensor_tensor(e[:], d[:], u[:], ALU.mult)
        nc.vector.tensor_tensor(hbuf[:, t + 1, :, :], c[:], e[:], ALU.add)
        nc.gpsimd.tensor_copy(flip[:, :, T - 1 - t, :], hbuf[:, t + 1, :, :])
        del pgr[t], pgu[t], pc[t]


def build_program():
    nc = bacc.Bacc("TRN2", target_bir_lowering=False, debug=False, num_devices=NC)
    ein = lambda n, s, d: nc.dram_tensor(n, s, d, kind="ExternalInput")
    xin = ein("xin", [BL, T, F], DT.float32)
    wgx0 = ein("wgx0", [P, 4, 4, P], DT.bfloat16)
    wcx0 = ein("wcx0", [P, 4, 2, P], DT.bfloat16)
    wgh0 = ein("wgh0", [P, 2, 4, P], DT.bfloat16)
    wch0 = ein("wch0", [P, 2, 2, P], DT.bfloat16)
    bg0 = ein("bg0", [P, 4], DT.float32)
    bc0 = ein("bc0", [P, 2], DT.float32)
    wgx1 = ein("wgx1", [P, 4, 4, P], DT.bfloat16)
    wcx1 = ein("wcx1", [P, 4, 2, P], DT.bfloat16)
    wgh1 = ein("wgh1", [P, 2, 4, P], DT.bfloat16)
    wch1 = ein("wch1", [P, 2, 2, P], DT.bfloat16)
    bg1 = ein("bg1", [P, 4], DT.float32)
    bc1 = ein("bc1", [P, 2], DT.float32)
    woutp = ein("woutp", [P, 4, V], DT.bfloat16)
    identp = ein("identp", [P, P], DT.bfloat16)
    feats_out = nc.dram_tensor("feats_out", [BH, T, 2 * U], DT.float32,
                               kind="ExternalOutput")
    dense_out = nc.dram_tensor("dense_out", [BH, T, V], DT.float32,
                               kind="ExternalOutput")

    with tile.TileContext(nc) as tc:
        with tc.tile_pool(name="wp", bufs=1) as wp, \
             tc.tile_pool(name="gxcp", bufs=1) as gxcp, \
             tc.tile_pool(name="h0p", bufs=1) as h0p, \
             tc.tile_pool(name="work", bufs=4) as work, \
             tc.tile_pool(name="dramp", bufs=1, space="DRAM") as dramp:
            # persistent weights
            wgx0t = wp.tile([P, 4, 4, P], DT.bfloat16)
            wcx0t = wp.tile([P, 4, 2, P], DT.bfloat16)
            wgh0t = wp.tile([P, 2, 4, P], DT.bfloat16)
            wch0t = wp.tile([P, 2, 2, P], DT.bfloat16)
            wgx1t = wp.tile([P, 4, 4, P], DT.bfloat16)
            wcx1t = wp.tile([P, 4, 2, P], DT.bfloat16)
            wgh1t = wp.tile([P, 2, 4, P], DT.bfloat16)
            wch1t = wp.tile([P, 2, 2, P], DT.bfloat16)
            bg0t = wp.tile([P, 4], DT.float32)
            bc0t = wp.tile([P, 2], DT.float32)
            bg1t = wp.tile([P, 4], DT.float32)
            bc1t = wp.tile([P, 2], DT.float32)
            idt = wp.tile([P, P], DT.bfloat16)
            for dst, src in ((wgx0t, wgx0), (wcx0t, wcx0), (wgh0t, wgh0),
                             (wch0t, wch0), (wgx1t, wgx1), (wcx1t, wcx1),
                             (wgh1t, wgh1), (wch1t, wch1), (bg0t, bg0),
                             (bc0t, bc0), (bg1t, bg1), (bc1t, bc1),
                             (idt, identp)):
                nc.sync.dma_start(dst[:], src[:])

            gxc = gxcp.tile([P, T, 6, BL], DT.bfloat16)   # shared L0/L1
            hbuf0 = h0p.tile([P, T + 1, 2, BL], DT.bfloat16)

            rank = nc.sync.cc_rank(GROUPS)

            # ---- phase 0: transpose x to xT [P, 4, T, BL] bf16 ----
            with tc.tile_pool(name="xtp", bufs=1) as xtp, \
                 tc.tile_pool(name="fl0p", bufs=1) as fl0p:
                flip0 = fl0p.tile([P, 2, T, BL], DT.bfloat16)
                xT = xtp.tile([P, 4, T, BL], DT.bfloat16)
                with tc.tile_pool(name="xload", bufs=4) as xload, \
                     tc.tile_pool(name="pst", bufs=4, space="PSUM") as pst:
                    for ex in range(BL):
                        for tt in range(4):
                            for ff in range(4):
                                xf = xload.tile([P, P], DT.float32, tag="xf",
                                                name=f"xf{ex}_{tt}_{ff}")
                                nc.sync.dma_start(
                                    xf[:], xin[ex, tt * P:(tt + 1) * P,
                                               ff * P:(ff + 1) * P])
                                xb = xload.tile([P, P], DT.bfloat16, tag="xb",
                                                name=f"xb{ex}_{tt}_{ff}")
                                nc.vector.tensor_copy(xb[:], xf[:])
                                pt = pst.tile([P, P], DT.bfloat16, tag="pt",
                                              name=f"pt{ex}_{tt}_{ff}")
                                nc.tensor.transpose(pt[:], xb[:], idt[:])
                                nc.vector.tensor_copy(
                                    xT[:, ff, tt * P:(tt + 1) * P, ex], pt[:])

                # ---- phase 1: L0 x-projections ----
                with tc.tile_pool(name="psx0", bufs=2, space="PSUM") as psx0:
                    rhs0 = [
                        (lambda k: (lambda t0: xT[:, k, t0:t0 + 64, :]))(k)
                        for k in range(4)
                    ]
                    _emit_xproj(nc, gxc, rhs0, wgx0t, wcx0t, bg0t, bc0t, psx0)

                # ---- phase 2: L0 scan ----
                with tc.tile_pool(name="psr0", bufs=3, space="PSUM") as psr, \
                     tc.tile_pool(name="psu0", bufs=3, space="PSUM") as psu, \
                     tc.tile_pool(name="psc0", bufs=2, space="PSUM") as psc:
                    _emit_scan(nc, "a", gxc, hbuf0, flip0, wgh0t, wch0t, idt,
                               psr, psu, psc, work)

                # ---- phase 3: exchange L0 ----
                cc0_in = dramp.tile([P, 2, T, BL], DT.bfloat16)
                cc0_out = dramp.tile([2, P, 2, T, BL], DT.bfloat16)
                nc.sync.dma_start(cc0_in[:], flip0[:])
                nc.gpsimd.collective_compute(
                    "AllGather", ALU.bypass, replica_groups=GROUPS,
                    ins=[cc0_in[:]], outs=[cc0_out[:]])

            with tc.tile_pool(name="r0p", bufs=1) as r0p, \
                 tc.tile_pool(name="h1p", bufs=1) as h1p:
                recv0 = r0p.tile([P, 2, T, BL], DT.bfloat16)
                with tc.If(rank < 1) as cmp:
                    nc.sync.dma_start(recv0[:], cc0_out[1])
                with cmp.Else():
                    nc.sync.dma_start(recv0[:], cc0_out[0])

                hbuf1 = h1p.tile([P, T + 1, 2, BL], DT.bfloat16)

                # ---- phase 4: L1 x-projections ----
                with tc.tile_pool(name="psx1", bufs=2, space="PSUM") as psx1:
                    def mk_rhs1(k):
                        if k < 2:
                            return lambda t0: hbuf0[:, 1 + t0:1 + t0 + 64, k, :]
                        return lambda t0: recv0[:, k - 2, t0:t0 + 64, :]
                    rhs1 = [mk_rhs1(k) for k in range(4)]
                    _emit_xproj(nc, gxc, rhs1, wgx1t, wcx1t, bg1t, bc1t, psx1)

                # ---- phase 5: L1 scan ----
                with tc.tile_pool(name="fl1p", bufs=1) as fl1p:
                    flip1 = fl1p.tile([P, 2, T, BL], DT.bfloat16)
                    with tc.tile_pool(name="psr1", bufs=3, space="PSUM") as psr, \
                         tc.tile_pool(name="psu1", bufs=3, space="PSUM") as psu, \
                         tc.tile_pool(name="psc1", bufs=2, space="PSUM") as psc:
                        _emit_scan(nc, "b", gxc, hbuf1, flip1, wgh1t, wch1t,
                                   idt, psr, psu, psc, work)

                    # ---- phase 6: exchange L1 ----
                    cc1_in = dramp.tile([P, 2, T, BL], DT.bfloat16)
                    cc1_out = dramp.tile([2, P, 2, T, BL], DT.bfloat16)
                    nc.sync.dma_start(cc1_in[:], flip1[:])
                    nc.gpsimd.collective_compute(
                        "AllGather", ALU.bypass, replica_groups=GROUPS,
                        ins=[cc1_in[:]], outs=[cc1_out[:]])

                with tc.tile_pool(name="r1p", bufs=1) as r1p, \
                     tc.tile_pool(name="stgp", bufs=1) as stgp:
                    recv1 = r1p.tile([P, 2, T, BL], DT.bfloat16)
                    stage = stgp.tile([P, 4, T, BH], DT.bfloat16)
                    with tc.If(rank < 1) as cmp:
                        nc.sync.dma_start(recv1[:], cc1_out[1])
                        for k in (0, 1):
                            nc.sync.dma_start(
                                stage[:, k, :, :], hbuf1[:, 1:, k, 0:BH])
                        for k in (0, 1):
                            nc.sync.dma_start(
                                stage[:, 2 + k, :, :], recv1[:, k, :, 0:BH])
                    with cmp.Else():
                        nc.sync.dma_start(recv1[:], cc1_out[0])
                        for k in (0, 1):
                            nc.sync.dma_start(
                                stage[:, k, :, :], hbuf1[:, 1:, k, BH:BL])
                        for k in (0, 1):
                            nc.sync.dma_start(
                                stage[:, 2 + k, :, :], recv1[:, k, :, BH:BL])

                    # ---- phase 7: feats output (transpose stage) ----
                    with tc.tile_pool(name="ftp", bufs=4, space="PSUM") as ftp, \
                         tc.tile_pool(name="fsb", bufs=4) as fsb:
                        for ex in range(BH):
                            for kt in range(4):
                                for tt in range(4):
                                    pt = ftp.tile([P, P], DT.bfloat16, tag="fpt",
                                                  name=f"fpt{ex}_{kt}_{tt}")
                                    nc.tensor.transpose(
                                        pt[:],
                                        stage[:, kt, tt * P:(tt + 1) * P, ex],
                                        idt[:])
                                    sb = fsb.tile([P, P], DT.float32, tag="fsb",
                                                  name=f"fsb{ex}_{kt}_{tt}")
                                    nc.vector.tensor_copy(sb[:], pt[:])
                                    nc.sync.dma_start(
                                        feats_out[ex, tt * P:(tt + 1) * P,
                                                  kt * P:(kt + 1) * P], sb[:])

                    # ---- phase 8: dense ----
                    with tc.tile_pool(name="wob", bufs=2) as wob, \
                         tc.tile_pool(name="pd", bufs=6, space="PSUM") as pd, \
                         tc.tile_pool(name="dsb", bufs=4) as dsb:
                        for vb in range(2):
                            vts = list(range(vb * 6, min(vb * 6 + 6, 12)))
                            v0b = vb * 6 * 512
                            nv_blk = sum(VT[vt] for vt in vts)
                            wo = wob.tile([P, 4, 3072], DT.bfloat16,
                                          tag="wo", name=f"wo{vb}")
                            for k in range(4):
                                nc.sync.dma_start(
                                    wo[:, k, 0:nv_blk],
                                    woutp[:, k, v0b:v0b + nv_blk])
                            for mt in range(16):
                                # M-tile = 32 timesteps x 4 examples
                                t0 = mt * 32
                                pss = {}
                                for k in range(4):
                                    off = 0
                                    for vt in vts:
                                        nv = VT[vt]
                                        if k == 0:
                                            pss[vt] = pd.tile(
                                                [P, 512], DT.float32, tag="pd",
                                                name=f"pd{vb}_{mt}_{vt}")
                                        nc.tensor.matmul(
                                            pss[vt][:, 0:nv],
                                            stage[:, k, t0:t0 + 32, :],
                                            wo[:, k, off:off + nv],
                                            start=(k == 0), stop=(k == 3))
                                        off += nv
                                for vt in vts:
                                    nv = VT[vt]
                                    v0 = vt * 512
                                    sb = dsb.tile([P, 512], DT.float32,
                                                  tag="dsb",
                                                  name=f"dsb{vb}_{mt}_{vt}")
                                    nc.vector.tensor_copy(sb[:, 0:nv],
                                                          pss[vt][:, 0:nv])
                                    nc.sync.dma_start(
                                        dense_out[:, t0:t0 + 32,
                                                  v0:v0 + nv].transpose(
                                                      [1, 0, 2]),
                                        sb[:, 0:nv])
    nc.compile()
    return nc


def _pack_w(w, nk, nm):
    """w [nk*128, nm*128] -> [P, nk, nm, P]"""
    return np.ascontiguousarray(
        w.reshape(nk, P, nm, P).transpose(1, 0, 2, 3)).astype(bf)


def _pack_bias(b, nm):
    return np.ascontiguousarray(b.reshape(nm, P).T).astype(np.float32)


def _prep_inputs(x, Wg, bg, Wc, bc, Wout, bout):
    """Build the 8 per-core input maps."""
    in_maps = []
    ident = np.eye(P).astype(bf)
    for c in range(NC):
        d, s = c // 4, c % 4
        ex = slice(s * BL, (s + 1) * BL)
        xs = x[ex]
        if d == 1:
            xs = xs[:, ::-1, :]
        m = {"xin": np.ascontiguousarray(xs, np.float32), "identp": ident}
        # layer 0
        Wg0, Wc0 = Wg[0, d], Wc[0, d]
        m["wgx0"] = _pack_w(Wg0[:F], 4, 4)
        m["wgh0"] = _pack_w(Wg0[F:], 2, 4)
        m["wcx0"] = _pack_w(Wc0[:F], 4, 2)
        m["wch0"] = _pack_w(Wc0[F:], 2, 2)
        m["bg0"] = _pack_bias(bg[0, d], 4)
        m["bc0"] = _pack_bias(bc[0, d], 2)
        # layer 1: x rows own-first
        Wg1, Wc1 = Wg[1, d], Wc[1, d]
        own, oth = slice(d * U, (d + 1) * U), slice((1 - d) * U, (2 - d) * U)
        Wg1x = np.concatenate([Wg1[:F][own], Wg1[:F][oth]], 0)
        Wc1x = np.concatenate([Wc1[:F][own], Wc1[:F][oth]], 0)
        m["wgx1"] = _pack_w(Wg1x, 4, 4)
        m["wgh1"] = _pack_w(Wg1[F:], 2, 4)
        m["wcx1"] = _pack_w(Wc1x, 4, 2)
        m["wch1"] = _pack_w(Wc1[F:], 2, 2)
        m["bg1"] = _pack_bias(bg[1, d], 4)
        m["bc1"] = _pack_bias(bc[1, d], 2)
        # Wout rows own-first
        Wo = Wout if d == 0 else np.concatenate([Wout[U:], Wout[:U]], 0)
        m["woutp"] = np.ascontiguousarray(
            Wo.reshape(4, P, V).transpose(1, 0, 2)).astype(bf)
        in_maps.append(m)
    return in_maps


def kernel(x, Wg, bg, Wc, bc, Wout, bout, training):
    global LAST_EXEC_NS
    x = np.asarray(x, np.float32)
    Wg = np.asarray(Wg, np.float32)
    bg = np.asarray(bg, np.float32)
    Wc = np.asarray(Wc, np.float32)
    bc = np.asarray(bc, np.float32)
    Wout = np.asarray(Wout, np.float32)
    bout = np.asarray(bout, np.float32)

    if "nc" not in _CACHE:
        _CACHE["nc"] = build_program()
    nc = _CACHE["nc"]
    in_maps = _prep_inputs(x, Wg, bg, Wc, bc, Wout, bout)
    trace = bool(int(os.environ.get("KERNEL_TRACE", "0")))
    if trace:
        sys.path.insert(0, os.path.dirname(os.path.abspath(__file__)))
        import axon_prof  # noqa: F401
    r = run_bass_kernel_spmd(nc, in_maps, list(range(NC)), trace=trace)
    LAST_EXEC_NS = r.exec_time_ns

    feats = np.zeros((B, T, 2 * U), np.float32)
    out = np.zeros((B, T, V), np.float32)
    for c in range(NC):
        d, s = c // 4, c % 4
        fo = r.results[c]["feats_out"]    # [4, T, 512] local-time, own-first
        do = r.results[c]["dense_out"]    # [4, T, V]
        if d == 0:
            exs = range(s * BL, s * BL + BH)
            for i, e in enumerate(exs):
                feats[e] = fo[i]
                out[e] = do[i]
        else:
            exs = range(s * BL + BH, s * BL + BL)
            for i, e in enumerate(exs):
                feats[e] = np.concatenate(
                    [fo[i, ::-1, U:], fo[i, ::-1, :U]], axis=-1)
                out[e] = do[i, ::-1]
    out += bout
    return feats, out


# revision 6
# speedup vs baseline: 1.2019x; 1.0884x over previous
"""Trainium2 Bass kernel for a 2-layer bidirectional GRU + dense vocab head.

Problem shapes (hardcoded):
  x [32, 512, 512] f32 -> feats [32, 512, 512] f32, out [32, 512, 6000] f32

Distribution across 8 NeuronCores: direction-parallel x batch-parallel.
  core c: direction d = c // 4, batch shard s = c % 4 (examples 8s..8s+8).
  Each core runs its direction's GRU chains for its 8 examples in a
  feature-transposed layout (feature on partitions, batch on free dim).
  Between layers, forward/backward partner cores exchange their (time-
  flipped) layer outputs with a pairwise AllGather; a runtime branch on
  the collective rank picks the partner's buffer. The final dense is
  split: rank-0 cores compute local examples 0:4, rank-1 cores 4:8.
  Time reversal for backward cores is handled on the host (inputs fed
  time-flipped, outputs flipped back).

All matmuls run in bf16 (f32 PSUM accumulation); GRU state is bf16;
gate preactivation x-parts are folded into PSUM via identity matmuls.
"""
import os
import sys

sys.path.insert(0, "/opt/trn_rl_repo")

import numpy as np
import ml_dtypes

import concourse.bacc as bacc
import concourse.mybir as mybir
import concourse.tile as tile
from concourse.bass_utils import run_bass_kernel_spmd

bf = ml_dtypes.bfloat16
DT = mybir.dt
AF = mybir.ActivationFunctionType
ALU = mybir.AluOpType

P = 128
B, T, F, U, V = 32, 512, 512, 256, 6000
BL = 8            # examples per core
BH = 4            # dense half per core
NC = 8
GROUPS = [[0, 4], [1, 5], [2, 6], [3, 7]]
VT = [512] * 11 + [368]   # V tiles

_CACHE = {}
LAST_EXEC_NS = None


def _emit_xproj(nc, gxc, rhs_tiles, wgx, wcx, bgt, bct, psx):
    """x-projections for one layer into gxc [P, T, 6, BL] bf16.

    rhs_tiles: list of 4 callables k -> AP [P, 64, BL] for a given t0 block.
    wgx [P, 4, 4, P], wcx [P, 4, 2, P]; biases [P, 4], [P, 2].
    """
    nblk = T // 64
    for blk in range(nblk):
        t0 = blk * 64
        for m in range(6):
            ps = psx.tile([P, 64, BL], DT.float32, tag="psx", name=f"psx_{id(gxc)}_{blk}_{m}")
            for k in range(4):
                w = wgx[:, k, m, :] if m < 4 else wcx[:, k, m - 4, :]
                nc.tensor.matmul(ps[:], w, rhs_tiles[k](t0),
                                 start=(k == 0), stop=(k == 3))
            bias = bgt[:, m:m + 1] if m < 4 else bct[:, m - 4:m - 3]
            nc.scalar.activation(gxc[:, t0:t0 + 64, m, :], ps[:], AF.Identity,
                                 bias=bias)


def _emit_scan(nc, lbl, gxc, hbuf, flip, wgh, wch, idt, psr, psu, psc, work):
    """v5 GRU scan. gxc [P,T,6,BL] bf16; hbuf [P,T+1,2,BL]; flip [P,2,T,BL]."""
    nc.vector.memset(hbuf[:, 0, :, :], 0.0)
    pgr = {}
    pgu = {}
    pc = {}

    def imm_r(t):
        tl = psr.tile([P, 2, BL], DT.float32, tag="pgr", name=f"pgr{lbl}_{t}")
        pgr[t] = tl
        nc.tensor.matmul(tl[:], idt[:], gxc[:, t, 0:2, :], start=True,
                         stop=False, skip_group_check=True)

    def imm_u(t):
        tl = psu.tile([P, 2, BL], DT.float32, tag="pgu", name=f"pgu{lbl}_{t}")
        pgu[t] = tl
        nc.tensor.matmul(tl[:], idt[:], gxc[:, t, 2:4, :], start=True,
                         stop=False, skip_group_check=True)

    def imm_c(t):
        tl = psc.tile([P, 2, BL], DT.float32, tag="pc", name=f"pc{lbl}_{t}")
        pc[t] = tl
        nc.tensor.matmul(tl[:], idt[:], gxc[:, t, 4:6, :], start=True,
                         stop=False, skip_group_check=True)

    imm_r(0); imm_u(0); imm_c(0)
    for t in range(T):
        if t + 1 < T:
            imm_r(t + 1); imm_u(t + 1)
        for m in (0, 1):
            for k in (0, 1):
                nc.tensor.matmul(pgr[t][:, m, :], wgh[:, k, m, :],
                                 hbuf[:, t, k, :], start=False,
                                 stop=(k == 1), skip_group_check=True)
        for m in (0, 1):
            for k in (0, 1):
                nc.tensor.matmul(pgu[t][:, m, :], wgh[:, k, m + 2, :],
                                 hbuf[:, t, k, :], start=False,
                                 stop=(k == 1), skip_group_check=True)
        ru_r = work.tile([P, 2, BL], DT.bfloat16, tag="ru_r", name=f"rur{lbl}_{t}")
        nc.scalar.activation(ru_r[:], pgr[t][:], AF.Sigmoid)
        rh = work.tile([P, 2, BL], DT.bfloat16, tag="rh", name=f"rh{lbl}_{t}")
        nc.vector.tensor_tensor(rh[:], ru_r[:], hbuf[:, t, :, :], ALU.mult)
        if t + 1 < T:
            imm_c(t + 1)
        for m in (0, 1):
            for k in (0, 1):
                nc.tensor.matmul(pc[t][:, m, :], wch[:, k, m, :],
                                 rh[:, k, :], start=False,
                                 stop=(k == 1), skip_group_check=True)
        u = work.tile([P, 2, BL], DT.float32, tag="u", name=f"u{lbl}_{t}")
        nc.scalar.activation(u[:], pgu[t][:], AF.Sigmoid)
        v = work.tile([P, 2, BL], DT.float32, tag="v", name=f"v{lbl}_{t}")
        nc.scalar.activation(v[:], pgu[t][:], AF.Sigmoid, scale=-1.0)
        tuh = work.tile([P, 2, BL], DT.float32, tag="tuh", name=f"tuh{lbl}_{t}")
        nc.vector.tensor_tensor(tuh[:], u[:], hbuf[:, t, :, :], ALU.mult)
        c = work.tile([P, 2, BL], DT.float32, tag="c", name=f"c{lbl}_{t}")
        nc.scalar.activation(c[:], pc[t][:], AF.Tanh)
        m1 = work.tile([P, 2, BL], DT.float32, tag="m1", name=f"m1{lbl}_{t}")
        nc.vector.tensor_tensor(m1[:], v[:], c[:], ALU.mult)
        nc.vector.tensor_tensor(hbuf[:, t + 1, :, :], m1[:], tuh[:], ALU.add)
        nc.gpsimd.tensor_copy(flip[:, :, T - 1 - t, :], hbuf[:, t + 1, :, :])
        del pgr[t], pgu[t], pc[t]


def build_program():
    nc = bacc.Bacc("TRN2", target_bir_lowering=False, debug=False, num_devices=NC)
    ein = lambda n, s, d: nc.dram_tensor(n, s, d, kind="ExternalInput")
    xin = ein("xin", [BL, T, F], DT.float32)
    wgx0 = ein("wgx0", [P, 4, 4, P], DT.bfloat16)
    wcx0 = ein("wcx0", [P, 4, 2, P], DT.bfloat16)
    wgh0 = ein("wgh0", [P, 2, 4, P], DT.bfloat16)
    wch0 = ein("wch0", [P, 2, 2, P], DT.bfloat16)
    bg0 = ein("bg0", [P, 4], DT.float32)
    bc0 = ein("bc0", [P, 2], DT.float32)
    wgx1 = ein("wgx1", [P, 4, 4, P], DT.bfloat16)
    wcx1 = ein("wcx1", [P, 4, 2, P], DT.bfloat16)
    wgh1 = ein("wgh1", [P, 2, 4, P], DT.bfloat16)
    wch1 = ein("wch1", [P, 2, 2, P], DT.bfloat16)
    bg1 = ein("bg1", [P, 4], DT.float32)
    bc1 = ein("bc1", [P, 2], DT.float32)
    woutp = ein("woutp", [P, 4, V], DT.bfloat16)
    identp = ein("identp", [P, P], DT.bfloat16)
    feats_out = nc.dram_tensor("feats_out", [BH, T, 2 * U], DT.float32,
                               kind="ExternalOutput")
    dense_out = nc.dram_tensor("dense_out", [BH, T, V], DT.float32,
                               kind="ExternalOutput")

    with tile.TileContext(nc) as tc:
        with tc.tile_pool(name="wp", bufs=1) as wp, \
             tc.tile_pool(name="gxcp", bufs=1) as gxcp, \
             tc.tile_pool(name="h0p", bufs=1) as h0p, \
             tc.tile_pool(name="work", bufs=4) as work, \
             tc.tile_pool(name="dramp", bufs=1, space="DRAM") as dramp:
            # persistent weights
            wgx0t = wp.tile([P, 4, 4, P], DT.bfloat16)
            wcx0t = wp.tile([P, 4, 2, P], DT.bfloat16)
            wgh0t = wp.tile([P, 2, 4, P], DT.bfloat16)
            wch0t = wp.tile([P, 2, 2, P], DT.bfloat16)
            wgx1t = wp.tile([P, 4, 4, P], DT.bfloat16)
            wcx1t = wp.tile([P, 4, 2, P], DT.bfloat16)
            wgh1t = wp.tile([P, 2, 4, P], DT.bfloat16)
            wch1t = wp.tile([P, 2, 2, P], DT.bfloat16)
            bg0t = wp.tile([P, 4], DT.float32)
            bc0t = wp.tile([P, 2], DT.float32)
            bg1t = wp.tile([P, 4], DT.float32)
            bc1t = wp.tile([P, 2], DT.float32)
            idt = wp.tile([P, P], DT.bfloat16)
            for dst, src in ((wgx0t, wgx0), (wcx0t, wcx0), (wgh0t, wgh0),
                             (wch0t, wch0), (wgx1t, wgx1), (wcx1t, wcx1),
                             (wgh1t, wgh1), (wch1t, wch1), (bg0t, bg0),
                             (bc0t, bc0), (bg1t, bg1), (bc1t, bc1),
                             (idt, identp)):
                nc.sync.dma_start(dst[:], src[:])

            gxc = gxcp.tile([P, T, 6, BL], DT.bfloat16)   # shared L0/L1
            hbuf0 = h0p.tile([P, T + 1, 2, BL], DT.bfloat16)

            rank = nc.sync.cc_rank(GROUPS)

            # ---- phase 0: transpose x to xT [P, 4, T, BL] bf16 ----
            with tc.tile_pool(name="xtp", bufs=1) as xtp, \
                 tc.tile_pool(name="fl0p", bufs=1) as fl0p:
                flip0 = fl0p.tile([P, 2, T, BL], DT.bfloat16)
                xT = xtp.tile([P, 4, T, BL], DT.bfloat16)
                with tc.tile_pool(name="xload", bufs=4) as xload, \
                     tc.tile_pool(name="pst", bufs=4, space="PSUM") as pst:
                    for ex in range(BL):
                        for tt in range(4):
                            for ff in range(4):
                                xf = xload.tile([P, P], DT.float32, tag="xf",
                                                name=f"xf{ex}_{tt}_{ff}")
                                nc.sync.dma_start(
                                    xf[:], xin[ex, tt * P:(tt + 1) * P,
                                               ff * P:(ff + 1) * P])
                                xb = xload.tile([P, P], DT.bfloat16, tag="xb",
                                                name=f"xb{ex}_{tt}_{ff}")
                                nc.vector.tensor_copy(xb[:], xf[:])
                                pt = pst.tile([P, P], DT.bfloat16, tag="pt",
                                              name=f"pt{ex}_{tt}_{ff}")
                                nc.tensor.transpose(pt[:], xb[:], idt[:])
                                nc.vector.tensor_copy(
                                    xT[:, ff, tt * P:(tt + 1) * P, ex], pt[:])

                # ---- phase 1: L0 x-projections ----
                with tc.tile_pool(name="psx0", bufs=2, space="PSUM") as psx0:
                    rhs0 = [
                        (lambda k: (lambda t0: xT[:, k, t0:t0 + 64, :]))(k)
                        for k in range(4)
                    ]
                    _emit_xproj(nc, gxc, rhs0, wgx0t, wcx0t, bg0t, bc0t, psx0)

                # ---- phase 2: L0 scan ----
                with tc.tile_pool(name="psr0", bufs=3, space="PSUM") as psr, \
                     tc.tile_pool(name="psu0", bufs=3, space="PSUM") as psu, \
                     tc.tile_pool(name="psc0", bufs=2, space="PSUM") as psc:
                    _emit_scan(nc, "a", gxc, hbuf0, flip0, wgh0t, wch0t, idt,
                               psr, psu, psc, work)

                # ---- phase 3: exchange L0 ----
                cc0_in = dramp.tile([P, 2, T, BL], DT.bfloat16)
                cc0_out = dramp.tile([2, P, 2, T, BL], DT.bfloat16)
                nc.sync.dma_start(cc0_in[:], flip0[:])
                nc.gpsimd.collective_compute(
                    "AllGather", ALU.bypass, replica_groups=GROUPS,
                    ins=[cc0_in[:]], outs=[cc0_out[:]])

            with tc.tile_pool(name="r0p", bufs=1) as r0p, \
                 tc.tile_pool(name="h1p", bufs=1) as h1p:
                recv0 = r0p.tile([P, 2, T, BL], DT.bfloat16)
                with tc.If(rank < 1) as cmp:
                    nc.sync.dma_start(recv0[:], cc0_out[1])
                with cmp.Else():
                    nc.sync.dma_start(recv0[:], cc0_out[0])

                hbuf1 = h1p.tile([P, T + 1, 2, BL], DT.bfloat16)

                # ---- phase 4: L1 x-projections ----
                with tc.tile_pool(name="psx1", bufs=2, space="PSUM") as psx1:
                    def mk_rhs1(k):
                        if k < 2:
                            return lambda t0: hbuf0[:, 1 + t0:1 + t0 + 64, k, :]
                        return lambda t0: recv0[:, k - 2, t0:t0 + 64, :]
                    rhs1 = [mk_rhs1(k) for k in range(4)]
                    _emit_xproj(nc, gxc, rhs1, wgx1t, wcx1t, bg1t, bc1t, psx1)

                # ---- phase 5: L1 scan ----
                with tc.tile_pool(name="fl1p", bufs=1) as fl1p:
                    flip1 = fl1p.tile([P, 2, T, BL], DT.bfloat16)
                    with tc.tile_pool(name="psr1", bufs=3, space="PSUM") as psr, \
                         tc.tile_pool(name="psu1", bufs=3, space="PSUM") as psu, \
                         tc.tile_pool(name="psc1", bufs=2, space="PSUM") as psc:
                        _emit_scan(nc, "b", gxc, hbuf1, flip1, wgh1t, wch1t,
                                   idt, psr, psu, psc, work)

                    # ---- phase 6: exchange L1 ----
                    cc1_in = dramp.tile([P, 2, T, BL], DT.bfloat16)
                    cc1_out = dramp.tile([2, P, 2, T, BL], DT.bfloat16)
                    nc.sync.dma_start(cc1_in[:], flip1[:])
                    nc.gpsimd.collective_compute(
                        "AllGather", ALU.bypass, replica_groups=GROUPS,
                        ins=[cc1_in[:]], outs=[cc1_out[:]])

                with tc.tile_pool(name="r1p", bufs=1) as r1p, \
                     tc.tile_pool(name="stgp", bufs=1) as stgp:
                    recv1 = r1p.tile([P, 2, T, BL], DT.bfloat16)
                    stage = stgp.tile([P, 4, T, BH], DT.bfloat16)
                    with tc.If(rank < 1) as cmp:
                        nc.sync.dma_start(recv1[:], cc1_out[1])
                        for k in (0, 1):
                            nc.sync.dma_start(
                                stage[:, k, :, :], hbuf1[:, 1:, k, 0:BH])
                        for k in (0, 1):
                            nc.sync.dma_start(
                                stage[:, 2 + k, :, :], recv1[:, k, :, 0:BH])
                    with cmp.Else():
                        nc.sync.dma_start(recv1[:], cc1_out[0])
                        for k in (0, 1):
                            nc.sync.dma_start(
                                stage[:, k, :, :], hbuf1[:, 1:, k, BH:BL])
                        for k in (0, 1):
                            nc.sync.dma_start(
                                stage[:, 2 + k, :, :], recv1[:, k, :, BH:BL])

                    # ---- phase 7: feats output (transpose stage) ----
                    with tc.tile_pool(name="ftp", bufs=4, space="PSUM") as ftp, \
                         tc.tile_pool(name="fsb", bufs=4) as fsb:
                        for ex in range(BH):
                            for kt in range(4):
                                for tt in range(4):
                                    pt = ftp.tile([P, P], DT.bfloat16, tag="fpt",
                                                  name=f"fpt{ex}_{kt}_{tt}")
                                    nc.tensor.transpose(
                                        pt[:],
                                        stage[:, kt, tt * P:(tt + 1) * P, ex],
                                        idt[:])
                                    sb = fsb.tile([P, P], DT.float32, tag="fsb",
                                                  name=f"fsb{ex}_{kt}_{tt}")
                                    nc.vector.tensor_copy(sb[:], pt[:])
                                    nc.sync.dma_start(
                                        feats_out[ex, tt * P:(tt + 1) * P,
                                                  kt * P:(kt + 1) * P], sb[:])

                    # ---- phase 8: dense ----
                    with tc.tile_pool(name="wob", bufs=2) as wob, \
                         tc.tile_pool(name="pd", bufs=6, space="PSUM") as pd, \
                         tc.tile_pool(name="dsb", bufs=4) as dsb:
                        for vb in range(2):
                            vts = list(range(vb * 6, min(vb * 6 + 6, 12)))
                            v0b = vb * 6 * 512
                            nv_blk = sum(VT[vt] for vt in vts)
                            wo = wob.tile([P, 4, 3072], DT.bfloat16,
                                          tag="wo", name=f"wo{vb}")
                            for k in range(4):
                                nc.sync.dma_start(
                                    wo[:, k, 0:nv_blk],
                                    woutp[:, k, v0b:v0b + nv_blk])
                            for mt in range(16):
                                # M-tile = 32 timesteps x 4 examples
                                t0 = mt * 32
                                pss = {}
                                for k in range(4):
                                    off = 0
                                    for vt in vts:
                                        nv = VT[vt]
                                        if k == 0:
                                            pss[vt] = pd.tile(
                                                [P, 512], DT.float32, tag="pd",
                                                name=f"pd{vb}_{mt}_{vt}")
                                        nc.tensor.matmul(
                                            pss[vt][:, 0:nv],
                                            stage[:, k, t0:t0 + 32, :],
                                            wo[:, k, off:off + nv],
                                            start=(k == 0), stop=(k == 3))
                                        off += nv
                                for vt in vts:
                                    nv = VT[vt]
                                    v0 = vt * 512
                                    sb = dsb.tile([P, 512], DT.float32,
                                                  tag="dsb",
                                                  name=f"dsb{vb}_{mt}_{vt}")
                                    nc.vector.tensor_copy(sb[:, 0:nv],
                                                          pss[vt][:, 0:nv])
                                    nc.sync.dma_start(
                                        dense_out[:, t0:t0 + 32,
                                                  v0:v0 + nv].transpose(
                                                      [1, 0, 2]),
                                        sb[:, 0:nv])
    nc.compile()
    return nc


def _pack_w(w, nk, nm):
    """w [nk*128, nm*128] -> [P, nk, nm, P]"""
    return np.ascontiguousarray(
        w.reshape(nk, P, nm, P).transpose(1, 0, 2, 3)).astype(bf)


def _pack_bias(b, nm):
    return np.ascontiguousarray(b.reshape(nm, P).T).astype(np.float32)


def _prep_inputs(x, Wg, bg, Wc, bc, Wout, bout):
    """Build the 8 per-core input maps."""
    in_maps = []
    ident = np.eye(P).astype(bf)
    for c in range(NC):
        d, s = c // 4, c % 4
        ex = slice(s * BL, (s + 1) * BL)
        xs = x[ex]
        if d == 1:
            xs = xs[:, ::-1, :]
        m = {"xin": np.ascontiguousarray(xs, np.float32), "identp": ident}
        # layer 0
        Wg0, Wc0 = Wg[0, d], Wc[0, d]
        m["wgx0"] = _pack_w(Wg0[:F], 4, 4)
        m["wgh0"] = _pack_w(Wg0[F:], 2, 4)
        m["wcx0"] = _pack_w(Wc0[:F], 4, 2)
        m["wch0"] = _pack_w(Wc0[F:], 2, 2)
        m["bg0"] = _pack_bias(bg[0, d], 4)
        m["bc0"] = _pack_bias(bc[0, d], 2)
        # layer 1: x rows own-first
        Wg1, Wc1 = Wg[1, d], Wc[1, d]
        own, oth = slice(d * U, (d + 1) * U), slice((1 - d) * U, (2 - d) * U)
        Wg1x = np.concatenate([Wg1[:F][own], Wg1[:F][oth]], 0)
        Wc1x = np.concatenate([Wc1[:F][own], Wc1[:F][oth]], 0)
        m["wgx1"] = _pack_w(Wg1x, 4, 4)
        m["wgh1"] = _pack_w(Wg1[F:], 2, 4)
        m["wcx1"] = _pack_w(Wc1x, 4, 2)
        m["wch1"] = _pack_w(Wc1[F:], 2, 2)
        m["bg1"] = _pack_bias(bg[1, d], 4)
        m["bc1"] = _pack_bias(bc[1, d], 2)
        # Wout rows own-first
        Wo = Wout if d == 0 else np.concatenate([Wout[U:], Wout[:U]], 0)
        m["woutp"] = np.ascontiguousarray(
            Wo.reshape(4, P, V).transpose(1, 0, 2)).astype(bf)
        in_maps.append(m)
    return in_maps


def kernel(x, Wg, bg, Wc, bc, Wout, bout, training):
    global LAST_EXEC_NS
    x = np.asarray(x, np.float32)
    Wg = np.asarray(Wg, np.float32)
    bg = np.asarray(bg, np.float32)
    Wc = np.asarray(Wc, np.float32)
    bc = np.asarray(bc, np.float32)
    Wout = np.asarray(Wout, np.float32)
    bout = np.asarray(bout, np.float32)

    if "nc" not in _CACHE:
        _CACHE["nc"] = build_program()
    nc = _CACHE["nc"]
    in_maps = _prep_inputs(x, Wg, bg, Wc, bc, Wout, bout)
    trace = bool(int(os.environ.get("KERNEL_TRACE", "0")))
    if trace:
        sys.path.insert(0, os.path.dirname(os.path.abspath(__file__)))
        import axon_prof  # noqa: F401
    r = run_bass_kernel_spmd(nc, in_maps, list(range(NC)), trace=trace)
    LAST_EXEC_NS = r.exec_time_ns

    feats = np.zeros((B, T, 2 * U), np.float32)
    out = np.zeros((B, T, V), np.float32)
    for c in range(NC):
        d, s = c // 4, c % 4
        fo = r.results[c]["feats_out"]    # [4, T, 512] local-time, own-first
        do = r.results[c]["dense_out"]    # [4, T, V]
        if d == 0:
            exs = range(s * BL, s * BL + BH)
            for i, e in enumerate(exs):
                feats[e] = fo[i]
                out[e] = do[i]
        else:
            exs = range(s * BL + BH, s * BL + BL)
            for i, e in enumerate(exs):
                feats[e] = np.concatenate(
                    [fo[i, ::-1, U:], fo[i, ::-1, :U]], axis=-1)
                out[e] = do[i, ::-1]
    out += bout
    return feats, out


# revision 7
# speedup vs baseline: 1.2756x; 1.0613x over previous
"""Trainium2 Bass kernel for a 2-layer bidirectional GRU + dense vocab head.

Problem shapes (hardcoded):
  x [32, 512, 512] f32 -> feats [32, 512, 512] f32, out [32, 512, 6000] f32

Distribution across 8 NeuronCores: direction-parallel x batch-parallel.
  core c: direction d = c // 4, batch shard s = c % 4 (examples 8s..8s+8).
  Each core runs its direction's GRU chains for its 8 examples in a
  feature-transposed layout (feature on partitions, batch on free dim).
  Between layers, forward/backward partner cores exchange their (time-
  flipped) layer outputs with a pairwise AllGather; a runtime branch on
  the collective rank picks the partner's buffer. The final dense is
  split: rank-0 cores compute local examples 0:4, rank-1 cores 4:8.
  Time reversal for backward cores is handled on the host (inputs fed
  time-flipped, outputs flipped back).

All matmuls run in bf16 (f32 PSUM accumulation); GRU state is bf16;
gate preactivation x-parts are folded into PSUM via identity matmuls.
"""
import os
import sys

sys.path.insert(0, "/opt/trn_rl_repo")

import numpy as np
import ml_dtypes

import concourse.bacc as bacc
import concourse.mybir as mybir
import concourse.tile as tile
from concourse.bass_utils import run_bass_kernel_spmd

bf = ml_dtypes.bfloat16
DT = mybir.dt
AF = mybir.ActivationFunctionType
ALU = mybir.AluOpType

P = 128
B, T, F, U, V = 32, 512, 512, 256, 6000
BL = 8            # examples per core
BH = 4            # dense half per core
NC = 8
GROUPS = [[0, 4], [1, 5], [2, 6], [3, 7]]
VT = [512] * 11 + [368]   # V tiles

_CACHE = {}
LAST_EXEC_NS = None


def _emit_xproj(nc, gxc, rhs_tiles, wgx, wcx, bgt, bct, psx):
    """x-projections for one layer into gxc [P, T, 6, BL] bf16.

    rhs_tiles: list of 4 callables k -> AP [P, 64, BL] for a given t0 block.
    wgx [P, 4, 4, P], wcx [P, 4, 2, P]; biases [P, 4], [P, 2].
    """
    nblk = T // 64
    for blk in range(nblk):
        t0 = blk * 64
        for m in range(6):
            ps = psx.tile([P, 64, BL], DT.float32, tag="psx", name=f"psx_{id(gxc)}_{blk}_{m}")
            for k in range(4):
                w = wgx[:, k, m, :] if m < 4 else wcx[:, k, m - 4, :]
                nc.tensor.matmul(ps[:], w, rhs_tiles[k](t0),
                                 start=(k == 0), stop=(k == 3))
            bias = bgt[:, m:m + 1] if m < 4 else bct[:, m - 4:m - 3]
            nc.scalar.activation(gxc[:, t0:t0 + 64, m, :], ps[:], AF.Identity,
                                 bias=bias)


def _emit_scan(nc, lbl, gxc, hbuf, flip, wgh, wch, idt, psr, psu, psc, work):
    """v5 GRU scan. gxc [P,T,6,BL] bf16; hbuf [P,T+1,2,BL]; flip [P,2,T,BL]."""
    nc.vector.memset(hbuf[:, 0, :, :], 0.0)
    pgr = {}
    pgu = {}
    pc = {}

    def imm_r(t):
        tl = psr.tile([P, 2, BL], DT.float32, tag="pgr", name=f"pgr{lbl}_{t}")
        pgr[t] = tl
        nc.tensor.matmul(tl[:], idt[:], gxc[:, t, 0:2, :], start=True,
                         stop=False, skip_group_check=True)

    def imm_u(t):
        tl = psu.tile([P, 2, BL], DT.float32, tag="pgu", name=f"pgu{lbl}_{t}")
        pgu[t] = tl
        nc.tensor.matmul(tl[:], idt[:], gxc[:, t, 2:4, :], start=True,
                         stop=False, skip_group_check=True)

    def imm_c(t):
        tl = psc.tile([P, 2, BL], DT.float32, tag="pc", name=f"pc{lbl}_{t}")
        pc[t] = tl
        nc.tensor.matmul(tl[:], idt[:], gxc[:, t, 4:6, :], start=True,
                         stop=False, skip_group_check=True)

    imm_r(0); imm_u(0); imm_c(0)
    for t in range(T):
        if t + 1 < T:
            imm_r(t + 1); imm_u(t + 1)
        for m in (0, 1):
            for k in (0, 1):
                nc.tensor.matmul(pgr[t][:, m, :], wgh[:, k, m, :],
                                 hbuf[:, t, k, :], start=False,
                                 stop=(k == 1), skip_group_check=True)
        for m in (0, 1):
            for k in (0, 1):
                nc.tensor.matmul(pgu[t][:, m, :], wgh[:, k, m + 2, :],
                                 hbuf[:, t, k, :], start=False,
                                 stop=(k == 1), skip_group_check=True)
        ru_r = work.tile([P, 2, BL], DT.bfloat16, tag="ru_r", name=f"rur{lbl}_{t}")
        nc.scalar.activation(ru_r[:], pgr[t][:], AF.Sigmoid)
        rh = work.tile([P, 2, BL], DT.bfloat16, tag="rh", name=f"rh{lbl}_{t}")
        nc.vector.tensor_tensor(rh[:], ru_r[:], hbuf[:, t, :, :], ALU.mult)
        if t + 1 < T:
            imm_c(t + 1)
        for m in (0, 1):
            for k in (0, 1):
                nc.tensor.matmul(pc[t][:, m, :], wch[:, k, m, :],
                                 rh[:, k, :], start=False,
                                 stop=(k == 1), skip_group_check=True)
        u = work.tile([P, 2, BL], DT.float32, tag="u", name=f"u{lbl}_{t}")
        nc.scalar.activation(u[:], pgu[t][:], AF.Sigmoid)
        v = work.tile([P, 2, BL], DT.float32, tag="v", name=f"v{lbl}_{t}")
        nc.scalar.activation(v[:], pgu[t][:], AF.Sigmoid, scale=-1.0)
        tuh = work.tile([P, 2, BL], DT.float32, tag="tuh", name=f"tuh{lbl}_{t}")
        nc.vector.tensor_tensor(tuh[:], u[:], hbuf[:, t, :, :], ALU.mult)
        c = work.tile([P, 2, BL], DT.float32, tag="c", name=f"c{lbl}_{t}")
        nc.scalar.activation(c[:], pc[t][:], AF.Tanh)
        m1 = work.tile([P, 2, BL], DT.float32, tag="m1", name=f"m1{lbl}_{t}")
        nc.vector.tensor_tensor(m1[:], v[:], c[:], ALU.mult)
        nc.vector.tensor_tensor(hbuf[:, t + 1, :, :], m1[:], tuh[:], ALU.add)
        nc.gpsimd.tensor_copy(flip[:, :, T - 1 - t, :], hbuf[:, t + 1, :, :])
        del pgr[t], pgu[t], pc[t]


def build_program():
    nc = bacc.Bacc("TRN2", target_bir_lowering=False, debug=False, num_devices=NC)
    ein = lambda n, s, d: nc.dram_tensor(n, s, d, kind="ExternalInput")
    xin = ein("xin", [BL, T, F], DT.float32)
    wgx0 = ein("wgx0", [P, 4, 4, P], DT.bfloat16)
    wcx0 = ein("wcx0", [P, 4, 2, P], DT.bfloat16)
    wgh0 = ein("wgh0", [P, 2, 4, P], DT.bfloat16)
    wch0 = ein("wch0", [P, 2, 2, P], DT.bfloat16)
    bg0 = ein("bg0", [P, 4], DT.float32)
    bc0 = ein("bc0", [P, 2], DT.float32)
    wgx1 = ein("wgx1", [P, 4, 4, P], DT.bfloat16)
    wcx1 = ein("wcx1", [P, 4, 2, P], DT.bfloat16)
    wgh1 = ein("wgh1", [P, 2, 4, P], DT.bfloat16)
    wch1 = ein("wch1", [P, 2, 2, P], DT.bfloat16)
    bg1 = ein("bg1", [P, 4], DT.float32)
    bc1 = ein("bc1", [P, 2], DT.float32)
    woutp = ein("woutp", [P, 4, V], DT.bfloat16)
    identp = ein("identp", [P, P], DT.bfloat16)
    feats_out = nc.dram_tensor("feats_out", [BH, T, 2 * U], DT.float32,
                               kind="ExternalOutput")
    dense_out = nc.dram_tensor("dense_out", [BH, T, V], DT.float32,
                               kind="ExternalOutput")

    with tile.TileContext(nc) as tc:
        with tc.tile_pool(name="wp", bufs=1) as wp, \
             tc.tile_pool(name="gxcp", bufs=1) as gxcp, \
             tc.tile_pool(name="h0p", bufs=1) as h0p, \
             tc.tile_pool(name="work", bufs=4) as work, \
             tc.tile_pool(name="dramp", bufs=1, space="DRAM") as dramp:
            # persistent weights
            wgx0t = wp.tile([P, 4, 4, P], DT.bfloat16)
            wcx0t = wp.tile([P, 4, 2, P], DT.bfloat16)
            wgh0t = wp.tile([P, 2, 4, P], DT.bfloat16)
            wch0t = wp.tile([P, 2, 2, P], DT.bfloat16)
            wgx1t = wp.tile([P, 4, 4, P], DT.bfloat16)
            wcx1t = wp.tile([P, 4, 2, P], DT.bfloat16)
            wgh1t = wp.tile([P, 2, 4, P], DT.bfloat16)
            wch1t = wp.tile([P, 2, 2, P], DT.bfloat16)
            bg0t = wp.tile([P, 4], DT.float32)
            bc0t = wp.tile([P, 2], DT.float32)
            bg1t = wp.tile([P, 4], DT.float32)
            bc1t = wp.tile([P, 2], DT.float32)
            idt = wp.tile([P, P], DT.bfloat16)
            for dst, src in ((wgx0t, wgx0), (wcx0t, wcx0), (wgh0t, wgh0),
                             (wch0t, wch0), (wgx1t, wgx1), (wcx1t, wcx1),
                             (wgh1t, wgh1), (wch1t, wch1), (bg0t, bg0),
                             (bc0t, bc0), (bg1t, bg1), (bc1t, bc1),
                             (idt, identp)):
                nc.sync.dma_start(dst[:], src[:])

            gxc = gxcp.tile([P, T, 6, BL], DT.bfloat16)   # shared L0/L1
            hbuf0 = h0p.tile([P, T + 1, 2, BL], DT.bfloat16)

            rank = nc.sync.cc_rank(GROUPS)

            # ---- phase 0: transpose x to xT [P, 4, T, BL] bf16 ----
            with tc.tile_pool(name="xtp", bufs=1) as xtp, \
                 tc.tile_pool(name="fl0p", bufs=1) as fl0p:
                flip0 = fl0p.tile([P, 2, T, BL], DT.bfloat16)
                xT = xtp.tile([P, 4, T, BL], DT.bfloat16)
                with tc.tile_pool(name="xload", bufs=4) as xload, \
                     tc.tile_pool(name="pst", bufs=4, space="PSUM") as pst:
                    for ex in range(BL):
                        for tt in range(4):
                            for ff in range(4):
                                xf = xload.tile([P, P], DT.float32, tag="xf",
                                                name=f"xf{ex}_{tt}_{ff}")
                                nc.sync.dma_start(
                                    xf[:], xin[ex, tt * P:(tt + 1) * P,
                                               ff * P:(ff + 1) * P])
                                xb = xload.tile([P, P], DT.bfloat16, tag="xb",
                                                name=f"xb{ex}_{tt}_{ff}")
                                nc.vector.tensor_copy(xb[:], xf[:])
                                pt = pst.tile([P, P], DT.bfloat16, tag="pt",
                                              name=f"pt{ex}_{tt}_{ff}")
                                nc.tensor.transpose(pt[:], xb[:], idt[:])
                                nc.vector.tensor_copy(
                                    xT[:, ff, tt * P:(tt + 1) * P, ex], pt[:])

                # ---- phase 1: L0 x-projections ----
                with tc.tile_pool(name="psx0", bufs=2, space="PSUM") as psx0:
                    rhs0 = [
                        (lambda k: (lambda t0: xT[:, k, t0:t0 + 64, :]))(k)
                        for k in range(4)
                    ]
                    _emit_xproj(nc, gxc, rhs0, wgx0t, wcx0t, bg0t, bc0t, psx0)

                # ---- phase 2: L0 scan ----
                with tc.tile_pool(name="psr0", bufs=3, space="PSUM") as psr, \
                     tc.tile_pool(name="psu0", bufs=3, space="PSUM") as psu, \
                     tc.tile_pool(name="psc0", bufs=2, space="PSUM") as psc:
                    _emit_scan(nc, "a", gxc, hbuf0, flip0, wgh0t, wch0t, idt,
                               psr, psu, psc, work)

                # ---- phase 3: exchange L0 ----
                cc0_in = dramp.tile([P, 2, T, BL], DT.bfloat16)
                cc0_out = dramp.tile([2, P, 2, T, BL], DT.bfloat16)
                nc.sync.dma_start(cc0_in[:], flip0[:])
                nc.gpsimd.collective_compute(
                    "AllGather", ALU.bypass, replica_groups=GROUPS,
                    ins=[cc0_in[:]], outs=[cc0_out[:]])

            with tc.tile_pool(name="r0p", bufs=1) as r0p, \
                 tc.tile_pool(name="h1p", bufs=1) as h1p:
                recv0 = r0p.tile([P, 2, T, BL], DT.bfloat16)
                with tc.If(rank < 1) as cmp:
                    nc.sync.dma_start(recv0[:], cc0_out[1])
                with cmp.Else():
                    nc.sync.dma_start(recv0[:], cc0_out[0])

                hbuf1 = h1p.tile([P, T + 1, 2, BL], DT.bfloat16)

                # ---- phase 4: L1 x-projections ----
                with tc.tile_pool(name="psx1", bufs=2, space="PSUM") as psx1:
                    def mk_rhs1(k):
                        if k < 2:
                            return lambda t0: hbuf0[:, 1 + t0:1 + t0 + 64, k, :]
                        return lambda t0: recv0[:, k - 2, t0:t0 + 64, :]
                    rhs1 = [mk_rhs1(k) for k in range(4)]
                    _emit_xproj(nc, gxc, rhs1, wgx1t, wcx1t, bg1t, bc1t, psx1)

                # ---- phase 5: L1 scan ----
                with tc.tile_pool(name="fl1p", bufs=1) as fl1p:
                    flip1 = fl1p.tile([P, 2, T, BL], DT.bfloat16)
                    with tc.tile_pool(name="psr1", bufs=3, space="PSUM") as psr, \
                         tc.tile_pool(name="psu1", bufs=3, space="PSUM") as psu, \
                         tc.tile_pool(name="psc1", bufs=2, space="PSUM") as psc:
                        _emit_scan(nc, "b", gxc, hbuf1, flip1, wgh1t, wch1t,
                                   idt, psr, psu, psc, work)

                    # ---- phase 6: exchange L1 ----
                    cc1_in = dramp.tile([P, 2, T, BL], DT.bfloat16)
                    cc1_out = dramp.tile([2, P, 2, T, BL], DT.bfloat16)
                    nc.sync.dma_start(cc1_in[:], flip1[:])
                    nc.gpsimd.collective_compute(
                        "AllGather", ALU.bypass, replica_groups=GROUPS,
                        ins=[cc1_in[:]], outs=[cc1_out[:]])

                with tc.tile_pool(name="r1p", bufs=1) as r1p, \
                     tc.tile_pool(name="stgp", bufs=1) as stgp:
                    recv1 = r1p.tile([P, 2, T, BL], DT.bfloat16)
                    stage = stgp.tile([P, 4, T, BH], DT.bfloat16)
                    with tc.If(rank < 1) as cmp:
                        nc.sync.dma_start(recv1[:], cc1_out[1])
                    with cmp.Else():
                        nc.sync.dma_start(recv1[:], cc1_out[0])
                    rank_v = nc.vector.cc_rank(GROUPS)
                    with tc.If(rank_v < 1) as cmpv:
                        for k in (0, 1):
                            nc.vector.tensor_copy(
                                stage[:, k, :, :], hbuf1[:, 1:, k, 0:BH])
                        for k in (0, 1):
                            nc.vector.tensor_copy(
                                stage[:, 2 + k, :, :], recv1[:, k, :, 0:BH])
                    with cmpv.Else():
                        for k in (0, 1):
                            nc.vector.tensor_copy(
                                stage[:, k, :, :], hbuf1[:, 1:, k, BH:BL])
                        for k in (0, 1):
                            nc.vector.tensor_copy(
                                stage[:, 2 + k, :, :], recv1[:, k, :, BH:BL])

                    # ---- phase 7: feats output (transpose stage) ----
                    with tc.tile_pool(name="ftp", bufs=4, space="PSUM") as ftp, \
                         tc.tile_pool(name="fsb", bufs=4) as fsb:
                        for ex in range(BH):
                            for kt in range(4):
                                for tt in range(4):
                                    pt = ftp.tile([P, P], DT.bfloat16, tag="fpt",
                                                  name=f"fpt{ex}_{kt}_{tt}")
                                    nc.tensor.transpose(
                                        pt[:],
                                        stage[:, kt, tt * P:(tt + 1) * P, ex],
                                        idt[:])
                                    sb = fsb.tile([P, P], DT.float32, tag="fsb",
                                                  name=f"fsb{ex}_{kt}_{tt}")
                                    nc.vector.tensor_copy(sb[:], pt[:])
                                    nc.sync.dma_start(
                                        feats_out[ex, tt * P:(tt + 1) * P,
                                                  kt * P:(kt + 1) * P], sb[:])

                    # ---- phase 8: dense ----
                    with tc.tile_pool(name="wob", bufs=2) as wob, \
                         tc.tile_pool(name="pd", bufs=6, space="PSUM") as pd, \
                         tc.tile_pool(name="dsb", bufs=4) as dsb:
                        for vb in range(2):
                            vts = list(range(vb * 6, min(vb * 6 + 6, 12)))
                            v0b = vb * 6 * 512
                            nv_blk = sum(VT[vt] for vt in vts)
                            wo = wob.tile([P, 4, 3072], DT.bfloat16,
                                          tag="wo", name=f"wo{vb}")
                            for k in range(4):
                                nc.sync.dma_start(
                                    wo[:, k, 0:nv_blk],
                                    woutp[:, k, v0b:v0b + nv_blk])
                            for mt in range(16):
                                # M-tile = 32 timesteps x 4 examples
                                t0 = mt * 32
                                pss = {}
                                for k in range(4):
                                    off = 0
                                    for vt in vts:
                                        nv = VT[vt]
                                        if k == 0:
                                            pss[vt] = pd.tile(
                                                [P, 512], DT.float32, tag="pd",
                                                name=f"pd{vb}_{mt}_{vt}")
                                        nc.tensor.matmul(
                                            pss[vt][:, 0:nv],
                                            stage[:, k, t0:t0 + 32, :],
                                            wo[:, k, off:off + nv],
                                            start=(k == 0), stop=(k == 3))
                                        off += nv
                                for vt in vts:
                                    nv = VT[vt]
                                    v0 = vt * 512
                                    sb = dsb.tile([P, 512], DT.float32,
                                                  tag="dsb",
                                                  name=f"dsb{vb}_{mt}_{vt}")
                                    nc.vector.tensor_copy(sb[:, 0:nv],
                                                          pss[vt][:, 0:nv])
                                    nc.sync.dma_start(
                                        dense_out[:, t0:t0 + 32,
                                                  v0:v0 + nv].transpose(
                                                      [1, 0, 2]),
                                        sb[:, 0:nv])
    nc.compile()
    return nc


def _pack_w(w, nk, nm):
    """w [nk*128, nm*128] -> [P, nk, nm, P]"""
    return np.ascontiguousarray(
        w.reshape(nk, P, nm, P).transpose(1, 0, 2, 3)).astype(bf)


def _pack_bias(b, nm):
    return np.ascontiguousarray(b.reshape(nm, P).T).astype(np.float32)


def _prep_inputs(x, Wg, bg, Wc, bc, Wout, bout):
    """Build the 8 per-core input maps."""
    in_maps = []
    ident = np.eye(P).astype(bf)
    for c in range(NC):
        d, s = c // 4, c % 4
        ex = slice(s * BL, (s + 1) * BL)
        xs = x[ex]
        if d == 1:
            xs = xs[:, ::-1, :]
        m = {"xin": np.ascontiguousarray(xs, np.float32), "identp": ident}
        # layer 0
        Wg0, Wc0 = Wg[0, d], Wc[0, d]
        m["wgx0"] = _pack_w(Wg0[:F], 4, 4)
        m["wgh0"] = _pack_w(Wg0[F:], 2, 4)
        m["wcx0"] = _pack_w(Wc0[:F], 4, 2)
        m["wch0"] = _pack_w(Wc0[F:], 2, 2)
        m["bg0"] = _pack_bias(bg[0, d], 4)
        m["bc0"] = _pack_bias(bc[0, d], 2)
        # layer 1: x rows own-first
        Wg1, Wc1 = Wg[1, d], Wc[1, d]
        own, oth = slice(d * U, (d + 1) * U), slice((1 - d) * U, (2 - d) * U)
        Wg1x = np.concatenate([Wg1[:F][own], Wg1[:F][oth]], 0)
        Wc1x = np.concatenate([Wc1[:F][own], Wc1[:F][oth]], 0)
        m["wgx1"] = _pack_w(Wg1x, 4, 4)
        m["wgh1"] = _pack_w(Wg1[F:], 2, 4)
        m["wcx1"] = _pack_w(Wc1x, 4, 2)
        m["wch1"] = _pack_w(Wc1[F:], 2, 2)
        m["bg1"] = _pack_bias(bg[1, d], 4)
        m["bc1"] = _pack_bias(bc[1, d], 2)
        # Wout rows own-first
        Wo = Wout if d == 0 else np.concatenate([Wout[U:], Wout[:U]], 0)
        m["woutp"] = np.ascontiguousarray(
            Wo.reshape(4, P, V).transpose(1, 0, 2)).astype(bf)
        in_maps.append(m)
    return in_maps


def kernel(x, Wg, bg, Wc, bc, Wout, bout, training):
    global LAST_EXEC_NS
    x = np.asarray(x, np.float32)
    Wg = np.asarray(Wg, np.float32)
    bg = np.asarray(bg, np.float32)
    Wc = np.asarray(Wc, np.float32)
    bc = np.asarray(bc, np.float32)
    Wout = np.asarray(Wout, np.float32)
    bout = np.asarray(bout, np.float32)

    if "nc" not in _CACHE:
        _CACHE["nc"] = build_program()
    nc = _CACHE["nc"]
    in_maps = _prep_inputs(x, Wg, bg, Wc, bc, Wout, bout)
    trace = bool(int(os.environ.get("KERNEL_TRACE", "0")))
    if trace:
        sys.path.insert(0, os.path.dirname(os.path.abspath(__file__)))
        import axon_prof  # noqa: F401
    r = run_bass_kernel_spmd(nc, in_maps, list(range(NC)), trace=trace)
    LAST_EXEC_NS = r.exec_time_ns

    feats = np.zeros((B, T, 2 * U), np.float32)
    out = np.zeros((B, T, V), np.float32)
    for c in range(NC):
        d, s = c // 4, c % 4
        fo = r.results[c]["feats_out"]    # [4, T, 512] local-time, own-first
        do = r.results[c]["dense_out"]    # [4, T, V]
        if d == 0:
            exs = range(s * BL, s * BL + BH)
            for i, e in enumerate(exs):
                feats[e] = fo[i]
                out[e] = do[i]
        else:
            exs = range(s * BL + BH, s * BL + BL)
            for i, e in enumerate(exs):
                feats[e] = np.concatenate(
                    [fo[i, ::-1, U:], fo[i, ::-1, :U]], axis=-1)
                out[e] = do[i, ::-1]
    out += bout
    return feats, out
